# revision 32
# baseline (speedup 1.0000x reference)
"""Causal group-query attention on 8 trn2 NeuronCores.

Sharding: 2 batches x 4 KV-head groups = 8 cores. Each core computes, for its
(batch b, group g): q/k/v projections for the group's 4 query heads + 1 KV
head, causal attention, and a partial output projection against the group's
512 rows of wo. Host sums the 4 group partials per batch (f16 partials).

On-chip layout (fp16 on the PE, fp32 PSUM accumulation):
  xT  [128(Hp), 16(Ho), T]   <- DMA-transpose of host-cast x16
  qT  [128(d), 4(h), T]      <- wq.T @ xT   (per-head slices)
  kT  [128(d), T]            <- wk.T @ xT
  vT  [128(d), T]            <- wv.T @ xT; SBUF->SBUF DMA transpose to
  v   [128(Ts), 4(tl), 128(d)] natural layout for the PV matmul
  scoresT [128(Ts), 2, Tq]   = kT_slice.T @ qT_slice     (PSUM f32, 2 banks)
  probsT  = exp(scale*scoresT)  one ACT instr per 2-block pair -> fp16;
     causal triangle of diag blocks zeroed by a DVE 0/1-mask multiply
  oT  [128(d), Tq] += v_s.T @ probsT ; den[128, Tq] += ones.T @ probsT
  out partial f16 = (oT/den).T @ wo_shard  (DVE drains only; ACT = exp only)

v5 vs v2 baseline (-9..-11% per-iteration, same-session A/B): HW
microbenching showed ACT exp costs ~598ns fixed + 0.68ns/col (one pair-exp
1294ns ~= the pair's 1280ns of PE work) and that cross-engine handoffs cost
~200ns beyond the cost model, so the exp->consume edge must be covered by
~2x its latency in PE work. Changes, in measured order of importance:
 - depth-2 software pipeline in the attention loop: scores+exp of pair i+2
   are issued before den/oT of pair i (sc PSUM slots are freed by the exp
   READ, so 2 pair slots suffice), giving each exp ~5us of PE cover and
   covering the den/oT bank WAR on the previous phase's recip+mult.
 - global feed: every pair gap gets an independent PE filler unit -- next
   chunk's projection groups (c=0/2) or o-projection n-slices of the
   previous chunk (split into 2-matmul halves for the 8-gap c=3 phases).
 - causal masking moved off the PE (was accumulate-matmuls) onto the idle
   Pool engine as a 0/1-triangle multiply on the probs, keeping the DVE
   queue free of ACT-dependent head-of-line blocking.
 - v-proj recast as wv-stationary 512-col matmuls + SBUF->SBUF DMA
   transpose; o-projection drains all on DVE (ACT runs exps only); adjacent
   o-proj n-slices share one ob tile -> half the out DMAs at 2x size;
   chunk-0/1 x-transposes for the next iteration issue at the body tail.
PSUM: 2x scores pairs (4 banks) + 2 mm + oT + den = 8 banks exactly.
"""

import numpy as np

import concourse.bass as bass
import concourse.mybir as mybir
import concourse.tile as tile
from concourse import bacc
from concourse.bass_utils import run_bass_kernel_spmd

# Problem shapes (hardcoded per contract)
B = 2
T = 2048
H = 2048
NH = 16
NKV = 4
D = 128  # head dim
G = NKV  # groups = cores per batch
HPG = NH // NKV  # 4 query heads per group
DQ = HPG * D  # 512 q dims per group
P = 128
KO = H // P  # 16 contraction subtiles
TCH = 512  # T chunk
NCH = T // TCH  # 4
NT = T // P  # 16
F16 = mybir.dt.float16
F32 = mybir.dt.float32
SCALE = float(1.0 / np.sqrt(D))

AF = mybir.ActivationFunctionType
ALU = mybir.AluOpType

# experiment toggles (defaults = shipped config; test.py may override)
FEED = True  # interleave oproj units into attention pair gaps
# timing probe ladder: 0=off, 1=den/oT read a constant instead of probs,
# 2=also skip exp+mask (no ACT traffic), 3=also skip x-transpose + out DMAs
NODEP = 0
# causal-mask placement: 0=DVE right after exp, 1=DVE just before consume,
# 2=Pool engine right after exp
MASKMODE = 2
# attention software-pipeline depth (1 or 2)
DEPTH = 2


def build_nc(reps: int = 1, staggered: bool = True):
    nc = bacc.Bacc(
        "TRN2",
        target_bir_lowering=False,
        debug=False,
        enable_asserts=False,
        num_devices=8,
    )
    x16 = nc.dram_tensor("x16", [T, H], F16, kind="ExternalInput").ap()
    wq16 = nc.dram_tensor("wq16", [P, KO, DQ], F16, kind="ExternalInput").ap()
    wk16 = nc.dram_tensor("wk16", [P, KO, D], F16, kind="ExternalInput").ap()
    wv16 = nc.dram_tensor("wv16", [P, KO, D], F16, kind="ExternalInput").ap()
    wo16 = nc.dram_tensor("wo16", [P, HPG, H], F16, kind="ExternalInput").ap()
    bq16 = nc.dram_tensor("bq16", [P, HPG], F16, kind="ExternalInput").ap()
    onem = nc.dram_tensor("onem", [P, P], F16, kind="ExternalInput").ap()
    tri01m = nc.dram_tensor("tri01m", [P, P], F16, kind="ExternalInput").ap()
    out = nc.dram_tensor("out", [T, H], F16, kind="ExternalOutput").ap()

    with tile.TileContext(nc) as tc:
        with (
            tc.tile_pool(name="const", bufs=1) as cp,
            tc.tile_pool(name="pers", bufs=1) as pp,
            tc.tile_pool(name="probs", bufs=5) as prp,
            tc.tile_pool(name="bcast", bufs=2) as bcp,
            tc.tile_pool(name="outb", bufs=4) as obp,
            tc.tile_pool(name="mmps", bufs=2, space="PSUM") as mm_ps,
            tc.tile_pool(name="scps", bufs=2, space="PSUM") as sc_psp,
            tc.tile_pool(name="otps", bufs=1, space="PSUM") as ot_psp,
            tc.tile_pool(name="denps", bufs=1, space="PSUM") as den_psp,
        ):
            # ---- persistent SBUF residents ----
            wq_sb = cp.tile([P, KO, DQ], F16)
            wk_sb = cp.tile([P, KO, D], F16)
            wv_sb = cp.tile([P, KO, D], F16)
            wo_sb = cp.tile([P, HPG, H], F16)
            bq_sb = cp.tile([P, HPG], F16)
            onem_sb = cp.tile([P, P], F16)
            tri01_sb = cp.tile([P, P], F16)

            dummy_pr = (
                cp.tile([P, 2, TCH], F16, name="dummy_pr") if NODEP else None
            )
            if NODEP:
                nc.vector.memset(dummy_pr[:], 0.001)

            xT_c = [pp.tile([P, KO, TCH], F16, name=f"xT{c}") for c in range(NCH)]
            qT_t = [
                [pp.tile([P, TCH], F16, name=f"qT{h}_{c}") for c in range(NCH)]
                for h in range(HPG)
            ]
            kT_t = [pp.tile([P, TCH], F16, name=f"kT{c}") for c in range(NCH)]
            vT_t = [pp.tile([P, TCH], F16, name=f"vT{c}") for c in range(NCH)]
            v4_t = [pp.tile([P, 4, D], F16, name=f"v4_{c}") for c in range(NCH)]
            oT_t = [
                [pp.tile([P, TCH], F16, name=f"oT{h}_{c}") for c in range(NCH)]
                for h in range(HPG)
            ]

            def transpose_chunk(c):
                if NODEP >= 3:
                    return
                for tl in range(4):
                    tb = c * 4 + tl
                    nc.sync.dma_start_transpose(
                        xT_c[c][:, :, tl * P : (tl + 1) * P],
                        x16[tb * P : (tb + 1) * P, :],
                    )

            def proj_group(c, kind, m=0):
                # one projection group (16 accumulating matmuls + drain) as a
                # feedable PE unit
                def emit():
                    xc = xT_c[c]
                    ps = mm_ps.tile([P, TCH], F32, name="mm", tag="mm")
                    if kind == "v":
                        for k in range(KO):
                            nc.tensor.matmul(
                                ps[:], wv_sb[:, k, :], xc[:, k, :],
                                start=(k == 0), stop=(k == KO - 1),
                            )
                        nc.vector.tensor_copy(vT_t[c][:], ps[:])
                        nc.sync.dma_start_transpose(v4_t[c][:], vT_t[c][:])
                    elif kind == "k":
                        for k in range(KO):
                            nc.tensor.matmul(
                                ps[:], wk_sb[:, k, :], xc[:, k, :],
                                start=(k == 0), stop=(k == KO - 1),
                            )
                        nc.vector.tensor_copy(kT_t[c][:], ps[:])
                    else:  # q head m
                        for k in range(KO):
                            nc.tensor.matmul(
                                ps[:],
                                wq_sb[:, k, m * P : (m + 1) * P],
                                xc[:, k, :],
                                start=(k == 0), stop=(k == KO - 1),
                            )
                        nc.vector.tensor_tensor(
                            qT_t[m][c][:],
                            ps[:],
                            bq_sb[:, m : m + 1].to_broadcast((P, TCH)),
                            ALU.add,
                        )

                return emit

            def proj_groups(c):
                return [proj_group(c, "v"), proj_group(c, "k")] + [
                    proj_group(c, "q", m) for m in range(HPG)
                ]

            def projections(c):
                for g in proj_groups(c):
                    g()

            def oproj_maker(cprev, tl):
                # yields per-n-slice units for t-block (cprev, tl): 4
                # accumulating matmuls + DVE drain into half of a paired ob
                # tile; one [128, 1024] DMA per n-pair. Used as PE filler
                # inside the attention pair loop.
                state = {}

                def unit(n):
                    def emit():
                        ps = mm_ps.tile([P, TCH], F32, name="mm", tag="mm")
                        for hh in range(HPG):
                            nc.tensor.matmul(
                                ps[:],
                                oT_t[hh][cprev][:, tl * P : (tl + 1) * P],
                                wo_sb[:, hh, n * TCH : (n + 1) * TCH],
                                start=(hh == 0), stop=(hh == HPG - 1),
                            )
                        if n % 2 == 0:
                            state["ob"] = obp.tile(
                                [P, 2, TCH], F16, name="ob", tag="ob"
                            )
                        ob = state["ob"]
                        nc.vector.tensor_copy(ob[:, n % 2, :], ps[:])
                        tb = cprev * 4 + tl
                        if n % 2 == 1 and NODEP < 3:
                            nc.sync.dma_start(
                                out[
                                    tb * P : (tb + 1) * P,
                                    (n - 1) * TCH : (n + 1) * TCH,
                                ],
                                ob[:],
                            )

                    return emit

                return unit

            def oproj_units(cprev, tl):
                u = oproj_maker(cprev, tl)
                return [u(n) for n in range(NCH)]

            def oproj_subunits(cprev, tl):
                # oproj units split into 2-matmul halves so all 8 pair gaps
                # of a c=3 attention phase get PE filler
                state = {}
                subs = []
                for n in range(NCH):
                    def s0(n=n):
                        ps = mm_ps.tile([P, TCH], F32, name="mm", tag="mm")
                        state[n] = ps
                        for hh in (0, 1):
                            nc.tensor.matmul(
                                ps[:],
                                oT_t[hh][cprev][:, tl * P : (tl + 1) * P],
                                wo_sb[:, hh, n * TCH : (n + 1) * TCH],
                                start=(hh == 0), stop=False,
                                skip_group_check=True,
                            )

                    def s1(n=n):
                        ps = state[n]
                        for hh in (2, 3):
                            nc.tensor.matmul(
                                ps[:],
                                oT_t[hh][cprev][:, tl * P : (tl + 1) * P],
                                wo_sb[:, hh, n * TCH : (n + 1) * TCH],
                                start=False, stop=(hh == 3),
                                skip_group_check=True,
                            )
                        if n % 2 == 0:
                            state["ob"] = obp.tile(
                                [P, 2, TCH], F16, name="ob", tag="ob"
                            )
                        ob = state["ob"]
                        nc.vector.tensor_copy(ob[:, n % 2, :], ps[:])
                        tb = cprev * 4 + tl
                        if n % 2 == 1 and NODEP < 3:
                            nc.sync.dma_start(
                                out[
                                    tb * P : (tb + 1) * P,
                                    (n - 1) * TCH : (n + 1) * TCH,
                                ],
                                ob[:],
                            )

                    subs += [s0, s1]
                return subs

            def attention(c, h, feed):
                # depth-1 software pipeline: scores+exp of pair i+1 are issued
                # before the den/oT consumes of pair i; one oproj unit per gap
                # keeps the PE ahead of the ACT exp chain.
                oT_ps = ot_psp.tile([P, TCH], F32, name="oT_ps", tag="oT")
                den_ps = den_psp.tile([P, TCH], F32, name="den_ps", tag="den")
                nsb = 4 * c + 4
                npair = nsb // 2
                pr_t = [None] * npair
                lo_t = [None] * npair

                def emit_sc(pi):
                    j0 = 2 * pi
                    los = [max(0, (j0 + idx) - 4 * c) * P for idx in range(2)]
                    lo_min = min(los)
                    sc = sc_psp.tile([P, 2, TCH], F32, name="sc", tag="sc")
                    pr = prp.tile([P, 2, TCH], F16, name="pr", tag="pr")
                    # diag blocks compute from the pair's lo_min so the exp
                    # below only reads written PSUM; the sub-lo sliver is
                    # never consumed and the in-block non-causal triangle is
                    # zeroed by the DVE mask multiply.
                    for idx in range(2):
                        j = j0 + idx
                        nc.tensor.matmul(
                            sc[:, idx, lo_min:],
                            kT_t[j // 4][:, (j % 4) * P : (j % 4 + 1) * P],
                            qT_t[h][c][:, lo_min:],
                            start=True, stop=True,
                        )
                    if NODEP < 2:
                        nc.scalar.activation(
                            pr[:, :, lo_min:], sc[:, :, lo_min:], AF.Exp,
                            scale=SCALE,
                        )
                        if MASKMODE != 1:
                            emit_masks(pi, pr, los)
                    pr_t[pi] = pr
                    lo_t[pi] = los

                def emit_masks(pi, pr, los):
                    eng = nc.gpsimd if MASKMODE == 2 else nc.vector
                    for idx in range(2):
                        jj = (2 * pi + idx) - 4 * c
                        if jj >= 0:
                            lo = los[idx]
                            eng.tensor_tensor(
                                pr[:, idx, lo : lo + P],
                                pr[:, idx, lo : lo + P],
                                tri01_sb[:],
                                ALU.mult,
                            )

                def emit_consume(pi):
                    pr = dummy_pr if NODEP else pr_t[pi]
                    if NODEP < 2 and MASKMODE == 1:
                        emit_masks(pi, pr_t[pi], lo_t[pi])
                    for idx in range(2):
                        j = 2 * pi + idx
                        lo = lo_t[pi][idx]
                        nc.tensor.matmul(
                            den_ps[:, lo:],
                            onem_sb[:],
                            pr[:, idx, lo:],
                            start=(j == 0), stop=(j == nsb - 1),
                            skip_group_check=True,
                        )
                        nc.tensor.matmul(
                            oT_ps[:, lo:],
                            v4_t[j // 4][:, j % 4, :],
                            pr[:, idx, lo:],
                            start=(j == 0), stop=(j == nsb - 1),
                            skip_group_check=True,
                        )

                # depth-2 software pipeline: sc slots are freed by the exp
                # read, so bufs=2 suffices; consume(pi) trails sc(pi+2),
                # giving ~5us of PE cover per exp and covering the den/oT
                # bank WAR on the previous phase's recip+mult at phase start.
                emit_sc(0)
                if DEPTH >= 2 and npair >= 2:
                    emit_sc(1)
                    if feed:
                        feed.pop(0)()
                for pi in range(npair):
                    if pi + DEPTH < npair:
                        emit_sc(pi + DEPTH)
                    if feed:
                        feed.pop(0)()
                    emit_consume(pi)
                bc32 = bcp.tile([P, TCH], F32, name="bc32", tag="bc")
                nc.vector.reciprocal(bc32[:], den_ps[:])
                nc.vector.tensor_tensor(oT_t[h][c][:], oT_ps[:], bc32[:], ALU.mult)

            def body(first=False):
                if not FEED:
                    if not first:
                        transpose_chunk(0)
                    for c in range(NCH):
                        if c + 1 < NCH:
                            transpose_chunk(c + 1)
                        projections(c)
                        for h in range(HPG):
                            attention(c, h, [])
                            if c > 0:
                                for u in oproj_units(c - 1, h):
                                    u()
                    for tl in range(4):
                        for u in oproj_units(NCH - 1, tl):
                            u()
                    return
                # global feed schedule: every attention pair gap gets PE
                # filler (next chunk's projection groups, oproj units).
                # chunk-0/1 x-transposes for this body were issued at the END
                # of the previous body (or before the loop for the first), so
                # projections(0) can start immediately.
                projections(0)
                pg1 = proj_groups(1)
                for h in range(HPG):
                    attention(0, h, [pg1[h]])  # v1 / k1 / q1m0 / q1m1
                transpose_chunk(2)
                pg1[4]()
                pg1[5]()
                for h in range(HPG):
                    attention(1, h, oproj_units(0, h))
                transpose_chunk(3)
                projections(2)
                pg3 = proj_groups(3)
                extras = [pg3[0:2], pg3[2:4], pg3[4:6], []]
                for h in range(HPG):
                    attention(
                        2, h,
                        oproj_units(1, h) + extras[h],
                    )
                for h in range(HPG):
                    attention(3, h, oproj_subunits(2, h))
                # next body's chunk-0/1 transposes overlap the oproj tail
                # and the loop barrier (xT_c[0/1] were last read during
                # projections of chunks 0/1 above)
                transpose_chunk(0)
                transpose_chunk(1)
                for tl in range(4):
                    for u in oproj_units(NCH - 1, tl):
                        u()

            if NODEP >= 3:
                for c in range(NCH):
                    nc.vector.memset(xT_c[c][:], 0.001)
            transpose_chunk(0)
            if FEED:
                transpose_chunk(1)
            nc.sync.dma_start(wq_sb[:], wq16)
            nc.sync.dma_start(wk_sb[:], wk16)
            nc.sync.dma_start(wv_sb[:], wv16)
            nc.sync.dma_start(bq_sb[:], bq16)
            nc.sync.dma_start(onem_sb[:], onem)
            nc.sync.dma_start(tri01_sb[:], tri01m)
            nc.sync.dma_start(wo_sb[:], wo16)
            if reps == 1:
                body(first=True)
            else:
                body(first=True)
                with tc.For_i(0, reps - 1, 1, staggered_reset=staggered):
                    body()

    nc.compile()
    return nc


def make_in_maps(x, wq, bq, wk, bk, wv, bv, wo):
    # bk shifts every score in a query row equally (softmax-invariant) and is
    # dropped; bv passes through softmax as a constant row handled on host.
    del bk, bv
    f16 = np.float16
    onem = np.ones((P, P), f16)
    tri01 = np.triu(np.ones((P, P), f16))  # keep q-offset >= s
    in_maps = []
    for core in range(8):
        b, g = divmod(core, G)
        wq_s = wq[:, g * DQ : (g + 1) * DQ].astype(f16)
        wk_s = wk[:, g * D : (g + 1) * D].astype(f16)
        wv_s = wv[:, g * D : (g + 1) * D].astype(f16)
        wo_s = wo[g * DQ : (g + 1) * DQ, :].astype(f16)
        in_maps.append(
            {
                "x16": np.ascontiguousarray(x[b].astype(f16)),
                "wq16": np.ascontiguousarray(
                    wq_s.reshape(KO, P, DQ).transpose(1, 0, 2)
                ),
                "wk16": np.ascontiguousarray(wk_s.reshape(KO, P, D).transpose(1, 0, 2)),
                "wv16": np.ascontiguousarray(wv_s.reshape(KO, P, D).transpose(1, 0, 2)),
                "wo16": np.ascontiguousarray(
                    wo_s.reshape(HPG, P, H).transpose(1, 0, 2)
                ),
                "bq16": np.ascontiguousarray(
                    bq[g * DQ : (g + 1) * DQ].astype(f16).reshape(HPG, P).T
                ),
                "onem": onem,
                "tri01m": tri01,
            }
        )
    return in_maps


_NC_CACHE = {}


def get_nc(reps: int = 1):
    if reps not in _NC_CACHE:
        _NC_CACHE[reps] = build_nc(reps)
    return _NC_CACHE[reps]


def kernel(x, wq, bq, wk, bk, wv, bv, wo):
    x, wq, bq, wk, bk, wv, bv, wo = (
        np.asarray(a, dtype=np.float32) for a in (x, wq, bq, wk, bk, wv, bv, wo)
    )
    nc = get_nc(1)
    in_maps = make_in_maps(x, wq, bq, wk, bk, wv, bv, wo)
    res = run_bass_kernel_spmd(nc, in_maps, core_ids=list(range(8)))
    out = np.zeros((B, T, H), np.float32)
    for core in range(8):
        b, _g = divmod(core, G)
        out[b] += res.results[core]["out"].astype(np.float32)
    # v-bias contribution: softmax rows sum to 1, so attn @ (1 x bv) = 1 x bv;
    # through the output projection that is repeat_kv(bv) @ wo added to every row.
    bv_rep = np.repeat(bv.reshape(NKV, D), HPG, axis=0).reshape(H)
    out += (bv_rep @ wo).reshape(1, 1, H)
    return out
